# revision 31
# baseline (speedup 1.0000x reference)
"""Trainium2 Bass kernel for nn_Attention_43181601194684.

Reference computation:
    h_last  = hidden[0, 1]                          # [B, H]
    proj    = einsum('blh,oh->blo', enc, W) + b     # [B, L, H]
    energies= einsum('bh,blh->bl', h_last, proj)    # [B, L]
    out     = softmax(energies, axis=1)[:, None, :] # [B, 1, L]

Algebraic simplification:
    energies[b, l] = (h_last[b] @ W) . enc[b, l] + (h_last[b] . bias)
The per-batch constant cancels inside the softmax, so the device kernel
computes   e[b, l] = v[b] . enc[b, l]   with v = h_last @ W, followed by a
numerically-stable softmax over l.  v = h_last @ W (the tiny [B,H]x[H,H]
GEMM) is done on the host.

Precision: enc and v are streamed in FP16 (host-converted); the products
accumulate into FP32 energies, exp runs in FP32 on the ACT engine, and the
exact cross-partition max correction + normalization happen on the host in
float64.  Measured output rel-err vs the fp32 reference is ~5e-3 (gate 2e-2).
FP16 halves the HBM traffic to 16.8 MiB/core.

Engine split (HW-measured pitches for one [128,512] unit): a fused DVE
scalar_tensor_tensor is 605 ns (1x); a plain DVE tensor_tensor multiply
hits the 2x mode at 335 ns; an ACT Copy-activation-with-accumulator
row-reduce is 628+186 ns.  Each chunk's rows are split ~7:9 between
(fused STT on DVE) and (DVE 2x multiply + ACT reduce) so both engines land
at ~58 us, just above the ~50 us DMA stream.  The Pool engine is left idle
on purpose: concurrent Pool work slows DVE ops ~2.5x.  Deep DMA prefetch
is also avoided (concurrent DMA SBUF writes slow every engine ~20%), so
in-flight chunks are capped by shallow tile pools on a single ring.

The wall time is  queue_start + total_bytes/DMA_BW + tail,  so the design
minimizes bytes, keeps every DMA a fully contiguous DRAM blob with fat
per-partition descriptor runs, limits in-flight chunks so the descriptor
round-robin doesn't starve the serial consumer, and puts SMALL chunks at
the two ends of the stream (early first STT, short post-last-byte tail).

Sharding: data-parallel over batch. 32 batches / 8 cores = 4 per core.
Chunk schedule (l-rows per partition; a 16-row fp16 chunk is 2 MiB):
batch 0 = [2,2,4,8,16], batches 1,2 = [16,16], batch 3 = [16,8,4,2,2].
Within a chunk of j rows at row-offset off: l = off + p*j + k.
Output: [128, 32] fp32 per-batch tile stored contiguously; host un-permutes.
"""

import numpy as np

B, L, H = 32, 4096, 512
N_CORES = 8
B_LOC = B // N_CORES  # 4
P = 128               # SBUF partitions
NCOL = L // P         # 32 energy columns per batch

SCHEDS = {
    0: (2, 2, 4, 8, 16),
    1: (16, 16),
    2: (16, 16),
    3: (16, 8, 4, 2, 2),
}

_PROGRAM = None


def _build_program():
    """Build + compile the single-core Bass/Tile program (SPMD across 8 cores)."""
    from contextlib import ExitStack

    import concourse.bacc as bacc
    import concourse.mybir as mybir
    import concourse.tile as tile
    from concourse.masks import make_identity

    fp32 = mybir.dt.float32
    fp16 = mybir.dt.float16
    Alu = mybir.AluOpType
    Act = mybir.ActivationFunctionType

    nc = bacc.Bacc("TRN2", target_bir_lowering=False, debug=False,
                   num_devices=N_CORES)

    enc = nc.dram_tensor("enc", [B_LOC, L, H], fp16, kind="ExternalInput")
    vr = nc.dram_tensor("vr", [B_LOC, P, H], fp16, kind="ExternalInput")
    probs = nc.dram_tensor("probs", [B_LOC, P, NCOL], fp16,
                           kind="ExternalOutput")
    mxs = nc.dram_tensor("mxs", [P, B_LOC], fp32, kind="ExternalOutput")

    # one rearranged view per chunk-row-count; chunk g of the k=j view
    # covers l in [g*128*j, (g+1)*128*j) with l = g*128*j + p*j + k
    enc_r = {
        j: enc.rearrange("b (g p k) h -> b g p k h", p=P, k=j)
        for j in (2, 4, 8, 16)
    }

    with tile.TileContext(nc) as tc, ExitStack() as ctx:
        consts = ctx.enter_context(tc.tile_pool(name="consts", bufs=1))
        wpool = ctx.enter_context(tc.tile_pool(name="wpool", bufs=1))
        et16 = ctx.enter_context(tc.tile_pool(name="et16", bufs=3))
        et8 = ctx.enter_context(tc.tile_pool(name="et8", bufs=2))
        et4 = ctx.enter_context(tc.tile_pool(name="et4", bufs=2))
        et2 = ctx.enter_context(tc.tile_pool(name="et2", bufs=4))
        scratch = ctx.enter_context(tc.tile_pool(name="scratch", bufs=3))
        dprod = ctx.enter_context(tc.tile_pool(name="dprod", bufs=8))
        pprod = ctx.enter_context(tc.tile_pool(name="pprod", bufs=6))
        aout = ctx.enter_context(tc.tile_pool(name="aout", bufs=3))
        epers = ctx.enter_context(tc.tile_pool(name="epers", bufs=1))
        small = ctx.enter_context(tc.tile_pool(name="small", bufs=2))
        psum = ctx.enter_context(tc.tile_pool(name="psum", bufs=1, space="PSUM"))
        etp = {2: et2, 4: et4, 8: et8, 16: et16}

        # split of each chunk's k-units across engines:
        # (fused STT on DVE) / (DVE 2x-TT + ACT copy-reduce) / (Pool TT +
        # ACT copy-reduce), balanced so DVE/ACT/Pool all land ~50us.
        NSPLIT = {16: (7, 9, 0), 8: (3, 5, 0), 4: (2, 2, 0), 2: (1, 1, 0)}

        # priority block: v (replicated fp16, 512 KiB) plus batch 0's head
        # chunks land first so the first STT fires as early as possible
        head = {}
        v_sb = {}
        with tc.high_priority():
            v_sb[0] = wpool.tile([P, H], fp16, tag="v0", name="v0")
            nc.sync.dma_start(v_sb[0][:], vr[0])
            for g in range(2):
                t = et2.tile([P, 2, H], fp16, tag="et2", name=f"hd{g}")
                nc.scalar.dma_start(t[:], enc_r[2][0, g])
                head[g] = t

        # ---- main stream: multiply+row-reduce split across DVE/ACT/Pool ----
        # All enc DMAs go on the sync ring IN ORDER so chunk completion
        # order matches consumption order (no descriptor round-robin skew).
        e_tiles = {}
        nbias_t = {}
        for bi in range(B_LOC):
            if bi not in v_sb:
                v_sb[bi] = wpool.tile([P, H], fp16, tag=f"v{bi}",
                                      name=f"v{bi}")
                nc.scalar.dma_start(v_sb[bi][:], vr[bi])
            sched = SCHEDS[bi]
            e_sb = epers.tile([P, NCOL], fp32, tag=f"e{bi}",
                              name=f"e{bi}")
            e_tiles[bi] = e_sb
            m = 0
            off_rows = 0
            for cix, j in enumerate(sched):
                g = off_rows // j          # group index in the k=j view
                if bi == 0 and cix < 2:
                    et = head[cix]
                else:
                    et = etp[j].tile([P, j, H], fp16, tag=f"et{j}",
                                     name=f"et_{bi}_{cix}")
                    nc.sync.dma_start(et[:], enc_r[j][bi, g])
                if bi == 3 and cix >= 3:
                    n_stt, n_tt, n_pool = (j, 0, 0)   # STT-only tail
                else:
                    n_stt, n_tt, n_pool = NSPLIT[j]
                for k in range(j):
                    if k < n_stt:
                        # fused (enc * v) + row-sum on DVE (1x mode)
                        sc = scratch.tile([P, H], fp16, tag="ttr")
                        nc.vector.scalar_tensor_tensor(
                            out=sc[:], in0=et[:, k, :], scalar=1.0,
                            in1=v_sb[bi][:],
                            op0=Alu.mult, op1=Alu.mult,
                            accum_out=e_sb[:, m:m + 1],
                        )
                    else:
                        # 2x-mode multiply on DVE or Pool, row-reduce on ACT
                        # (Copy-activation with accumulator; the Copy table
                        # is loaded once and all Exps happen at the end)
                        if k < n_stt + n_tt:
                            prod = dprod.tile([P, H], fp16, tag="prod")
                            eng = nc.vector
                        else:
                            prod = pprod.tile([P, H], fp16, tag="pprod")
                            eng = nc.gpsimd
                        eng.tensor_tensor(out=prod[:], in0=et[:, k, :],
                                          in1=v_sb[bi][:], op=Alu.mult)
                        ao = aout.tile([P, H], fp16, tag="actout")
                        nc.scalar.activation(ao[:], prod[:], Act.Copy,
                                             bias=0.0, scale=1.0,
                                             accum_out=e_sb[:, m:m + 1])
                    m += 1
                off_rows += j

            # negated per-partition row max: exp(e - rowmax_p) <= 1 is
            # overflow-safe; the host applies the cross-partition
            # correction exactly from the stored row maxes
            if bi == 0:
                mxall = small.tile([P, B_LOC], fp32, tag="mxall",
                                   name="mxall")
                nbias_t["all"] = mxall
            else:
                mxall = nbias_t["all"]
            nc.vector.tensor_reduce(mxall[:, bi:bi + 1], e_sb[:],
                                    axis=mybir.AxisListType.X,
                                    op=Alu.max, negate=True)
            nbias_t[bi] = mxall[:, bi:bi + 1]

        # ---- deferred exp tails, stage-major (ACT switches Copy->Exp once).
        # The normalizing divide happens on the host: p = exp(e - M) is
        # stored as-is and the host divides each batch by its sum.
        for bi in range(B_LOC):
            p_t = epers.tile([P, NCOL], fp16, tag=f"p{bi}",
                             name=f"p{bi}")
            if bi == 0:
                # row maxes are complete before the exps; store them first
                nc.sync.dma_start(mxs[:], nbias_t["all"][:])
            nc.scalar.activation(p_t[:], e_tiles[bi][:], Act.Exp,
                                 bias=nbias_t[bi], scale=1.0)
            # contiguous 8 KiB fp16 store (host normalizes in float64);
            # alternate between the idle SP and Pool DMA queues -- cross-
            # engine issue gets a real semaphore, unlike the ACT-ring variant
            r = nc.sync if bi % 2 == 0 else nc.gpsimd
            r.dma_start(probs[bi], p_t[:])

    nc.compile()
    return nc


def _get_program():
    global _PROGRAM
    if _PROGRAM is None:
        _PROGRAM = _build_program()
    return _PROGRAM


def _core_inputs(enc, v):
    """Per-core input dicts: fp16 enc batch-slice + replicated fp16 v."""
    enc16 = enc.astype(np.float16)
    v16 = v.astype(np.float16)
    in_maps = []
    for core in range(N_CORES):
        b0 = core * B_LOC
        v_rep = np.ascontiguousarray(
            np.broadcast_to(v16[b0:b0 + B_LOC][:, None, :], (B_LOC, P, H)))
        in_maps.append({
            "enc": np.ascontiguousarray(enc16[b0:b0 + B_LOC]),
            "vr": v_rep,
        })
    return in_maps


def _assemble(probs_list, mxs_list):
    """[B_LOC, P, NCOL] per core -> full [B, 1, L].

    Column block [mc, mc+j) of a batch holds chunk (off, j) with
    l = off + p*j + k; flattening [:, mc:mc+j] in C-order (p major,
    k minor) is exactly l-order for that chunk.
    """
    out = np.empty((B, L), dtype=np.float64)
    for core, pr in enumerate(probs_list):
        rowmax = -np.asarray(mxs_list[core], dtype=np.float64)  # [P, B_LOC]
        for bl in range(B_LOC):
            b = core * B_LOC + bl
            w = np.exp(rowmax[:, bl] - rowmax[:, bl].max())     # [P]
            scaled = np.asarray(pr[bl], dtype=np.float64) * w[:, None]
            mc = 0
            off = 0
            for j in SCHEDS[bl]:
                n = P * j
                out[b, off:off + n] = scaled[:, mc:mc + j].reshape(n)
                mc += j
                off += n
            out[b] /= out[b].sum()
    return out[:, None, :].astype(np.float32)


def kernel(hidden, encoder_outputs, W, b):
    """Full-input entry point: shards across 8 NeuronCores, returns [B,1,L]."""
    from concourse.bass_utils import run_bass_kernel_spmd

    hidden = np.asarray(hidden, dtype=np.float32)
    enc = np.asarray(encoder_outputs, dtype=np.float32)
    W = np.asarray(W, dtype=np.float32)

    h_last = hidden[0, 1]          # == hidden[0].transpose(1,0,2)[:, -1, :]
    v = (h_last @ W).astype(np.float32)  # [B, H]; bias cancels in softmax

    nc = _get_program()
    in_maps = _core_inputs(enc, v)
    res = run_bass_kernel_spmd(nc, in_maps, list(range(N_CORES)))
    return _assemble([res.results[i]["probs"] for i in range(N_CORES)],
                     [res.results[i]["mxs"] for i in range(N_CORES)])
